# revision 1
# baseline (speedup 1.0000x reference)
"""PointsRenderer (alpha compositing over K points/pixel) on 8 trn2 cores.

Sharding: data-parallel over batch B=8 -> 1 image per NeuronCore; the
[100000, 4] feature table is replicated per core and gathered with
per-partition indirect DMA (128 rows per call; on this hardware the
indirect DMA consumes exactly one offset per output partition, so the
gather is FT calls per tile).

Per-core layout: the 512*512*8 = 2^21 fragment stream is split over the
128 SBUF partitions (16384 fragments each), processed in NT tiles of FT
fragments per partition.  Compositing (weights, front-to-back
transmittance cumprod, contrib) streams on DVE/ACT and overlaps the
gather; the K-sum is a tree reduction feeding a compact output tile.
"""

import numpy as np

import concourse.bass as bass
import concourse.mybir as mybir
import concourse.tile as tile
from concourse import bacc
from concourse.bass_utils import run_bass_kernel_spmd

B, H, W, K, P, C = 8, 512, 512, 8, 100000, 4
NF = H * W * K          # fragments per core (B=1 shard)
PARTS = 128
PERPART = NF // PARTS   # 16384
FT = 256                # fragments per partition per tile (32K descriptors/gather call)
NT = PERPART // FT      # 16
PIX_T = FT // K         # 128 pixels per partition per tile

F32 = mybir.dt.float32
I32 = mybir.dt.int32


def build(inv_r2: float, idx_words: int):
    """idx_words: 2 when host idx is int64 (little-endian pairs), 1 for int32."""
    nc = bacc.Bacc(None, target_bir_lowering=False, debug=False)
    idx32 = nc.dram_tensor(
        "idx32", [PARTS, NT, FT * idx_words], I32, kind="ExternalInput"
    )
    d2 = nc.dram_tensor("d2", [PARTS, NT, FT], F32, kind="ExternalInput")
    feat = nc.dram_tensor("feat", [P, C], F32, kind="ExternalInput")
    out = nc.dram_tensor("out", [PARTS, NT, PIX_T * C], F32, kind="ExternalOutput")

    with tile.TileContext(nc) as tc:
        with tc.tile_pool(name="io", bufs=2) as io, \
             tc.tile_pool(name="gp", bufs=2) as gp, \
             tc.tile_pool(name="wp", bufs=2) as wp:
            for t in range(NT):
                ipair = io.tile([PARTS, FT * idx_words], I32, tag="ipair")
                nc.sync.dma_start(ipair[:], idx32[:, t, :])
                d2t = io.tile([PARTS, FT], F32, tag="d2t")
                nc.sync.dma_start(d2t[:], d2[:, t, :])

                if idx_words == 2:
                    # extract low 32-bit words of the little-endian int64 indices
                    ilow = io.tile([PARTS, FT], I32, tag="ilow")
                    nc.vector.tensor_copy(
                        ilow[:],
                        ipair[:].rearrange("p (f two) -> p f two", two=2)[:, :, 0],
                    )
                else:
                    ilow = ipair

                # gather feature rows: G[p, f*C:(f+1)*C] = feat[ilow[p, f], :]
                # HW indirect DMA consumes exactly one offset per output
                # partition, so issue FT calls of 128 rows each.
                G = gp.tile([PARTS, FT * C], F32, tag="G")
                for f in range(FT):
                    nc.gpsimd.indirect_dma_start(
                        out=G[:, f * C:(f + 1) * C],
                        out_offset=None,
                        in_=feat[:],
                        in_offset=bass.IndirectOffsetOnAxis(
                            ap=ilow[:, f:f + 1], axis=0
                        ),
                    )

                # alpha_k = 1 - d2*inv_r2 (ACT), om_k = d2*inv_r2 (DVE)
                alpha = wp.tile([PARTS, FT], F32, tag="alpha")
                nc.scalar.activation(
                    alpha[:], d2t[:], mybir.ActivationFunctionType.Copy,
                    bias=1.0, scale=-float(inv_r2),
                )
                om = wp.tile([PARTS, FT], F32, tag="om")
                nc.vector.tensor_scalar_mul(om[:], d2t[:], float(inv_r2))

                # contrib_k = alpha_k * prod_{j<k} om_j   (front-to-back)
                cb = wp.tile([PARTS, FT], F32, tag="cb")
                cbv = cb[:].rearrange("p (t k) -> p t k", k=K)
                alv = alpha[:].rearrange("p (t k) -> p t k", k=K)
                omv = om[:].rearrange("p (t k) -> p t k", k=K)
                rt = wp.tile([PARTS, PIX_T], F32, tag="rt")
                nc.vector.tensor_copy(cbv[:, :, 0], alv[:, :, 0])
                nc.vector.tensor_copy(rt[:], omv[:, :, 0])
                for k in range(1, K):
                    nc.vector.tensor_mul(cbv[:, :, k], alv[:, :, k], rt[:])
                    if k < K - 1:
                        nc.vector.tensor_mul(rt[:], rt[:], omv[:, :, k])

                # G *= contrib (broadcast over channel)
                G3 = G[:].rearrange("p (f c) -> p f c", c=C)
                nc.vector.tensor_mul(
                    G3, G3,
                    cb[:].rearrange("p (f one) -> p f one", one=1).to_broadcast([PARTS, FT, C]),
                )

                # sum over K: tree reduction, final into compact tile
                Gv = G[:].rearrange("p (t k c) -> p t k c", k=K, c=C)
                nc.vector.tensor_add(Gv[:, :, 0:4, :], Gv[:, :, 0:4, :], Gv[:, :, 4:8, :])
                nc.vector.tensor_add(Gv[:, :, 0:2, :], Gv[:, :, 0:2, :], Gv[:, :, 2:4, :])
                outT = wp.tile([PARTS, PIX_T, C], F32, tag="outT")
                nc.vector.tensor_add(outT[:], Gv[:, :, 0, :], Gv[:, :, 1, :])

                nc.sync.dma_start(out[:, t, :], outT[:].rearrange("p t c -> p (t c)"))

    nc.compile()
    return nc


last_result = None
last_nc = None
last_in_maps = None


def kernel(idx, dists2, features, radius):
    global last_result, last_nc, last_in_maps
    idx = np.ascontiguousarray(idx)
    dists2 = np.ascontiguousarray(dists2, dtype=np.float32)
    features = np.ascontiguousarray(features, dtype=np.float32)
    r = float(np.asarray(radius).reshape(-1)[0])
    inv_r2 = 1.0 / (r * r)

    if idx.dtype == np.int64:
        idx_words = 2
    else:
        idx = np.ascontiguousarray(idx, dtype=np.int32)
        idx_words = 1

    nc = build(inv_r2, idx_words)

    in_maps = []
    for b in range(B):
        idx32_b = idx[b].reshape(-1).view(np.int32).reshape(PARTS, NT, FT * idx_words)
        d2_b = dists2[b].reshape(PARTS, NT, FT)
        in_maps.append({"idx32": idx32_b, "d2": d2_b, "feat": features})

    last_nc, last_in_maps = nc, in_maps
    res = run_bass_kernel_spmd(nc, in_maps, core_ids=list(range(B)))
    last_result = res

    out = np.empty((B, H, W, C), dtype=np.float32)
    for b in range(B):
        out[b] = res.results[b]["out"].reshape(H, W, C)
    return out



# revision 2
# speedup vs baseline: 1.4840x; 1.4840x over previous
"""PointsRenderer (alpha compositing over K points/pixel) on 8 trn2 cores.

Gather strategy: batched SWDGE descriptor DMA (InstDMAGatherAnt,
single_packet=False, 4 SWDGE queues).  The [100000, 4] f32 feature table
is repacked on host into 256B stripes of 4 rows (tp[25000, 64] f32,
stripe s bytes [0,64) = rows 4s..4s+3).  Each fragment gathers its
64B stripe with one descriptor (stripe index < 25000 fits int16); the
final 16B row is picked on DVE by a 4-way masked select using o=idx%4,
fused with the compositing weight.

Per-core layout: B=8 -> 1 image/core; the 2^21 fragments split
partition-major over 128 partitions (16384 each), in 256 tiles of 64
fragments per partition (num_idxs=8192 per gather call).
"""

import numpy as np

import concourse.mybir as mybir
import concourse.tile as tile
from concourse import bacc, library_config
from concourse.bass_utils import run_bass_kernel_spmd

B, H, W, K, P, C = 8, 512, 512, 8, 100000, 4
NF = H * W * K            # fragments per core
PARTS = 128
PERPART = NF // PARTS     # 16384
FT = 64                   # fragments per partition per tile
NT = PERPART // FT        # 256
NI = PARTS * FT           # 8192 descriptors per gather call
SP = (P + 3) // 4         # 25000 stripes
NQ = 4                    # SWDGE queues

F32 = mybir.dt.float32
I16 = mybir.dt.int16


def _dma_gather_ant(nc, out_ap, in_ap, idxs_ap, num_idxs, elem_size,
                    elem_step, queue_num):
    """InstDMAGatherAnt with sub-256B payload (bass's dma_gather wrapper
    asserts elem_size_bytes%256==0; the ucode only needs the row stride
    to be 256B-granular)."""
    gp = nc.gpsimd
    _in_ap = gp.lower_ap_dma(in_ap, for_custom_bir_dma=True)
    _idxs_ap = gp.lower_ap(idxs_ap)
    _out_ap = gp.lower_ap(out_ap)
    stride_bytes = elem_step * 4
    assert stride_bytes % 256 == 0
    return gp.add_instruction(
        mybir.InstDMAGatherAnt(
            name=nc.get_next_instruction_name(),
            ins=[*_in_ap, _idxs_ap, gp.lower_val_access(gp.to_reg(num_idxs))],
            outs=[_out_ap],
            transpose=False,
            num_idxs=num_idxs,
            elem_size=elem_size,
            stride_bytes_256=stride_bytes // 256,
            gen_mode=0,
            single_packet=False,
            queue_num=queue_num,
            sbuf_tokens_per_rank=0,
            sbuf_free_dim_per_rank=0,
            sbuf_free_dim_pad_per_rank=0,
            sbuf_byte_offset=0,
        )
    )


def build(inv_r2: float):
    nc = bacc.Bacc(None, target_bir_lowering=False, debug=False,
                   num_swdge_queues=NQ)
    tp = nc.dram_tensor("tp", [SP, 64], F32, kind="ExternalInput")
    wi = nc.dram_tensor("wi", [NT, PARTS, NI // 16], I16, kind="ExternalInput")
    d2 = nc.dram_tensor("d2", [PARTS, NT, FT], F32, kind="ExternalInput")
    o4 = nc.dram_tensor("o4", [PARTS, NT, FT], F32, kind="ExternalInput")
    out = nc.dram_tensor("out", [PARTS, NT, (FT // K) * C], F32,
                         kind="ExternalOutput")

    with tile.TileContext(nc) as tc:
        with tc.tile_pool(name="io", bufs=3) as io, \
             tc.tile_pool(name="gp", bufs=3) as gpool, \
             tc.tile_pool(name="wp", bufs=3) as wp:
            nc.gpsimd.load_library(library_config.mlp)
            for t in range(NT):
                wit = io.tile([PARTS, NI // 16], I16, tag="wit")
                nc.sync.dma_start(wit[:], wi[t, :, :])
                d2t = io.tile([PARTS, FT], F32, tag="d2t")
                nc.sync.dma_start(d2t[:], d2[:, t, :])
                o4t = io.tile([PARTS, FT], F32, tag="o4t")
                nc.sync.dma_start(o4t[:], o4[:, t, :])

                # one descriptor per fragment: 64B stripe (4 rows)
                G = gpool.tile([PARTS, FT * 16], F32, tag="G")
                _dma_gather_ant(
                    nc, G[:].rearrange("p (j e) -> p j e", e=16),
                    tp[:, 0:16], wit[:], NI, 16, 64, queue_num=t % NQ)

                # alpha_k = 1 - d2*inv_r2 (ACT), om_k = d2*inv_r2 (DVE)
                alpha = wp.tile([PARTS, FT], F32, tag="alpha")
                nc.scalar.activation(
                    alpha[:], d2t[:], mybir.ActivationFunctionType.Copy,
                    bias=1.0, scale=-float(inv_r2))
                om = wp.tile([PARTS, FT], F32, tag="om")
                nc.vector.tensor_scalar_mul(om[:], d2t[:], float(inv_r2))

                # contrib_k = alpha_k * prod_{j<k} om_j (front-to-back)
                cb = wp.tile([PARTS, FT], F32, tag="cb")
                cbv = cb[:].rearrange("p (m k) -> p m k", k=K)
                alv = alpha[:].rearrange("p (m k) -> p m k", k=K)
                omv = om[:].rearrange("p (m k) -> p m k", k=K)
                rt = wp.tile([PARTS, FT // K], F32, tag="rt")
                nc.vector.tensor_copy(cbv[:, :, 0], alv[:, :, 0])
                nc.vector.tensor_copy(rt[:], omv[:, :, 0])
                for k in range(1, K):
                    nc.vector.tensor_mul(cbv[:, :, k], alv[:, :, k], rt[:])
                    if k < K - 1:
                        nc.vector.tensor_mul(rt[:], rt[:], omv[:, :, k])

                # 4-way stripe select fused with contrib: for q in 0..3
                #   CW_q = cb * (o4 == q);  acc += CW_q * G[:, :, 4q:4q+4]
                Gv = G[:].rearrange("p (j e) -> p j e", e=16)
                acc = []
                for q in range(4):
                    mq = wp.tile([PARTS, FT], F32, tag=f"m{q}")
                    nc.vector.tensor_scalar(
                        mq[:], o4t[:], float(q), None,
                        mybir.AluOpType.is_equal)
                    nc.vector.tensor_mul(mq[:], mq[:], cb[:])
                    aq = wp.tile([PARTS, FT, C], F32, tag=f"a{q}")
                    nc.vector.tensor_mul(
                        aq[:], Gv[:, :, 4 * q:4 * q + 4],
                        mq[:].rearrange("p (f one) -> p f one", one=1)
                            .to_broadcast([PARTS, FT, C]))
                    acc.append(aq)
                nc.vector.tensor_add(acc[0][:], acc[0][:], acc[1][:])
                nc.vector.tensor_add(acc[2][:], acc[2][:], acc[3][:])
                nc.vector.tensor_add(acc[0][:], acc[0][:], acc[2][:])

                # sum over K: tree reduction into compact tile
                Sv = acc[0][:].rearrange("p (m k) c -> p m k c", k=K)
                nc.vector.tensor_add(Sv[:, :, 0:4, :], Sv[:, :, 0:4, :],
                                     Sv[:, :, 4:8, :])
                nc.vector.tensor_add(Sv[:, :, 0:2, :], Sv[:, :, 0:2, :],
                                     Sv[:, :, 2:4, :])
                outT = wp.tile([PARTS, FT // K, C], F32, tag="outT")
                nc.vector.tensor_add(outT[:], Sv[:, :, 0, :], Sv[:, :, 1, :])

                nc.sync.dma_start(
                    out[:, t, :], outT[:].rearrange("p m c -> p (m c)"))

    nc.compile()
    return nc


def make_in_maps(idx, dists2, features):
    idx = np.ascontiguousarray(idx).astype(np.int64)
    dists2 = np.ascontiguousarray(dists2, dtype=np.float32)
    features = np.ascontiguousarray(features, dtype=np.float32)

    # stripe-packed table: stripe s = rows 4s..4s+3 in first 64B of 256B row
    tp = np.zeros((SP, 64), np.float32)
    flat = np.zeros((SP * 4, C), np.float32)
    flat[:P] = features
    tp[:, 0:16] = flat.reshape(SP, 16)

    stripe = (idx >> 2).astype(np.int16)      # [B,H,W,K] < 25000
    o4 = (idx & 3).astype(np.float32)

    # wrapped + group-replicated index list per tile
    i = np.arange(NI)
    A, Sl = i % 16, i // 16
    p, xo = i % 128, i // 128

    in_maps = []
    for b in range(B):
        s_b = stripe[b].reshape(PARTS, NT, FT)
        Wt = np.empty((NT, 16, NI // 16), np.int16)
        Wt[:, A, Sl] = s_b[p, :, xo].T        # [NT, 8192]
        wi_b = np.broadcast_to(
            Wt[:, None, :, :], (NT, 8, 16, NI // 16)
        ).reshape(NT, PARTS, NI // 16).copy()
        in_maps.append({
            "tp": tp,
            "wi": wi_b,
            "d2": dists2[b].reshape(PARTS, NT, FT),
            "o4": o4[b].reshape(PARTS, NT, FT),
        })
    return in_maps


def unshard_one(res_map):
    return res_map["out"].reshape(H, W, C)


def kernel(idx, dists2, features, radius):
    r = float(np.asarray(radius).reshape(-1)[0])
    nc = build(1.0 / (r * r))
    in_maps = make_in_maps(idx, dists2, features)
    res = run_bass_kernel_spmd(nc, in_maps, core_ids=list(range(B)))
    out = np.empty((B, H, W, C), dtype=np.float32)
    for b in range(B):
        out[b] = unshard_one(res.results[b])
    return out
